# revision 16
# baseline (speedup 1.0000x reference)
"""2-layer GCN (GCNConv without normalization) as a Bass/Trainium2 SPMD kernel on 8 NeuronCores.

Strategy (graph/data parallel, node sharding):
  - Nodes are sorted by in-degree and dealt round-robin to the 8 cores, so
    every core owns ~E/8 edges and windows of 128 nodes have near-uniform
    max in-degree (minimal gather padding).
  - Layer algebra: segment_sum commutes with the linear maps, so both layers
    aggregate 16-wide tables:
        L1: h1 = x @ W1 (16 wide); agg1 = scatter_add(w * h1[src]);
            a1 = relu(agg1 + b1)
        L2: agg2 = scatter_add(w * a1[src]); out = log_softmax(agg2 @ W2 + b2)
  - Edge gather: the 16-wide tables are viewed as 4-node-packed 256B rows
    ([R/4, 64] f32).  Each destination slot fetches its source's pack with a
    single dma_gather descriptor (int16 pack ids < 32768).  Each <=63-column
    chunk is issued as two half-gathers rotated over the 4 SWDGE queues with
    single_packet=False (~500 descriptors/ring keeps 2+ instructions in
    flight per ring; measured ~10x cheaper per edge than the per-column
    indirect_dma_start baseline, which pays ~1.3 us per 128 descriptors).
  - The per-edge weight is stored as a 4-wide one-hot (w at the source's
    position in its pack): one in-place DVE multiply + one strided
    tensor_reduce over 4*K[w] elements per 128-node window performs both the
    pack selection and the segment sum.
  - Gather padding: each destination window of 128 nodes is padded to the
    window's max degree; pad slots use pack 0 with all-zero w4.

The harness calls kernel(**inputs) with full inputs; sharding happens here.
"""

import os
import sys
import time

import numpy as np

sys.path.insert(0, "/opt/trn_rl_repo")

NCORES = 8
P = 128
NQ = 4           # SWDGE queues to rotate dma_gather over
CS_MAX = 63      # slot columns per chunk: 63*128 idx -> 505 descs/ring, 2 fit per 1024-ring

# stash for test harness introspection (exec time, etc.)
LAST = {}


def _host_prep(x, edge_index, edge_weight):
    """Build the sharded/padded data layout. Returns (cfg, per_core_arrays)."""
    N, Fin = x.shape
    E = edge_index.shape[1]
    src = np.ascontiguousarray(edge_index[0]).astype(np.int64)
    dst = np.ascontiguousarray(edge_index[1]).astype(np.int64)

    deg = np.bincount(dst, minlength=N)
    order = np.argsort(-deg, kind="stable")  # node ids, highest in-degree first
    rank_of = np.empty(N, np.int64)
    rank_of[order] = np.arange(N)
    core_of = rank_of % NCORES
    lrank = rank_of // NCORES

    NLOC = -(-N // NCORES)          # nodes per core (assumes N % NCORES == 0 for exactness)
    W = -(-NLOC // P)               # windows per core
    NPAD = W * P
    w_of = lrank // P
    p_of = lrank % P
    # gather-table row of each node (same layout for the h1 and a1 tables)
    tbl = (core_of * NPAD + p_of * W + w_of).astype(np.int64)

    # per-window max degree K[w]: window w covers global degree-ranks [w*1024, ...)
    K = np.zeros(W, np.int64)
    for w in range(W):
        lo = w * P * NCORES
        hi = min((w + 1) * P * NCORES, N)
        K[w] = deg[order[lo:hi]].max() if hi > lo else 0
    K = np.maximum(K, 1)  # keep windows non-empty so every reduce is well-formed
    OFF = np.concatenate([[0], np.cumsum(K)]).astype(np.int64)
    S_total = int(OFF[-1])

    # chunk windows so each chunk's gather fits one dma_gather instruction
    chunks = []
    w0, s0 = 0, 0
    for w in range(W):
        if s0 + K[w] > CS_MAX and w > w0:
            chunks.append((w0, w, int(OFF[w0]), int(s0)))
            w0, s0 = w, 0
        s0 += K[w]
    chunks.append((w0, W, int(OFF[w0]), int(s0)))

    # edge slots: sort edges by destination node id; j-th edge of node d goes
    # to (core_of[d], p_of[d], OFF[w_of[d]] + j)
    e_sort = np.argsort(dst, kind="stable")
    ds = dst[e_sort]
    ss = src[e_sort]
    ws = np.ascontiguousarray(edge_weight)[e_sort].astype(np.float32)
    starts = np.concatenate([[0], np.cumsum(deg)])
    j = np.arange(E) - starts[ds]
    col = OFF[w_of[ds]] + j

    # 4-node pack id + position-in-pack one-hot weights
    pack_arr = np.zeros((NCORES, P, S_total), np.int16)
    pack_arr[core_of[ds], p_of[ds], col] = (tbl[ss] // 4).astype(np.int16)
    w4_arr = np.zeros((NCORES, P, S_total, 4), np.float32)
    w4_arr[core_of[ds], p_of[ds], col, tbl[ss] % 4] = ws

    # idx16: gather position g = col*128 + p reads int16 idx at
    # [band*16 + g%16, g//16]; replicate into all 8 bands (HW reads a
    # queue-dependent band; CoreSim reads band 0).
    NCOLS = S_total * 8
    idx16 = np.empty((NCORES, P, NCOLS), np.int16)
    for k in range(NCORES):
        g_order = pack_arr[k].T.reshape(-1)            # g = col*128 + p
        band = g_order.reshape(NCOLS, 16).T            # [16, NCOLS]
        idx16[k] = np.tile(band, (8, 1))
    w4d = np.ascontiguousarray(w4_arr.reshape(NCORES, P, S_total * 4))

    # x^T shards in lrank order: core k, column l -> node with lrank l
    # (the h1_shard write AP maps window/partition back to table row p*W+w)
    node_of = order[: NLOC * NCORES].reshape(NLOC, NCORES)
    xts = np.zeros((NCORES, Fin, NPAD), np.float32)
    for k in range(NCORES):
        xts[k, :, :NLOC] = x[node_of[:, k]].T

    # output reshuffle: result row p*W+w of core k -> node node_of[w*128+p, k]
    l = np.arange(NLOC)
    out_rows = (l % P) * W + l // P  # row in kernel output for local slot l

    cfg = dict(
        N=N, Fin=Fin, E=E, NLOC=NLOC, W=W, NPAD=NPAD,
        K=K.tolist(), OFF=OFF.tolist(), S_total=S_total, chunks=chunks,
    )
    return cfg, idx16, w4d, xts, node_of, out_rows


def _build(nc, cfg, H, C):
    """Emit the Bass/Tile program (identical on all cores).

    GCN_STAGE env limits how much of the pipeline is emitted (debug bisect):
      h1 < ag1 < g1 < l1 < ag2 < l2 < full (default)
    """
    import concourse.mybir as mybir
    import concourse.tile as tile
    from concourse import bass
    from concourse.masks import make_identity

    Fin, W, NPAD, S_total = cfg["Fin"], cfg["W"], cfg["NPAD"], cfg["S_total"]
    K, OFF, chunks = cfg["K"], cfg["OFF"], cfg["chunks"]
    KB = Fin // P  # K-blocks for the x @ W1 matmul
    f32 = mybir.dt.float32

    xT = nc.dram_tensor("xT", [Fin, NPAD], f32, kind="ExternalInput")
    W1 = nc.dram_tensor("W1", [Fin, H], f32, kind="ExternalInput")
    W2 = nc.dram_tensor("W2", [H, C], f32, kind="ExternalInput")
    b1b = nc.dram_tensor("b1b", [P, H], f32, kind="ExternalInput")
    b2b = nc.dram_tensor("b2b", [P, C], f32, kind="ExternalInput")
    idx = nc.dram_tensor("idx", [P, S_total * 8], mybir.dt.int16, kind="ExternalInput")
    w4 = nc.dram_tensor("w4", [P, S_total * 4], f32, kind="ExternalInput")
    out = nc.dram_tensor("out", [NPAD, C], f32, kind="ExternalOutput")

    h1_shard = nc.dram_tensor("h1_shard", [NPAD, H], f32)
    h1_full = nc.dram_tensor("h1_full", [NCORES * NPAD, H], f32, addr_space="Shared")
    a1_shard = nc.dram_tensor("a1_shard", [NPAD, H], f32)
    a1_full = nc.dram_tensor("a1_full", [NCORES * NPAD, H], f32, addr_space="Shared")

    XTW = 4  # windows per xT load chunk
    STAGE = os.environ.get("GCN_STAGE", "full")
    ORDER = ["h1", "ag1", "g1", "l1", "ag2", "l2", "full"]
    lvl = ORDER.index(STAGE)

    CS_ALLOC = max(c[3] for c in chunks)  # largest chunk (a lone window may exceed CS_MAX)

    def bcast_mid(ap, n):
        """[P, F] -> [P, n, F] with a step-0 middle dim."""
        return bass.AP(ap.tensor, ap.offset, [list(ap.ap[0]), [0, n], list(ap.ap[1])])

    GBUFS = 9 if CS_ALLOC <= 66 else 5  # keep the gather pool inside SBUF
    with tile.TileContext(nc) as tc:
        with (
            tc.tile_pool(name="const", bufs=1) as constp,
            tc.tile_pool(name="persist", bufs=1) as persist,
            tc.tile_pool(name="xt", bufs=2) as xtp,
            tc.tile_pool(name="gath", bufs=GBUFS) as gathp,
            tc.tile_pool(name="idxm", bufs=6) as idxmp,
            tc.tile_pool(name="w4m", bufs=6) as w4mp,
            tc.tile_pool(name="gt", bufs=2) as gtp,
            tc.tile_pool(name="ps_h", bufs=2, space="PSUM") as ps_h,
            tc.tile_pool(name="ps_t", bufs=2, space="PSUM") as ps_t,
            tc.tile_pool(name="ps_o", bufs=2, space="PSUM") as ps_o,
        ):
            # constants
            w1_sb = constp.tile([P, KB * H], f32, tag="w1")
            for kb in range(KB):
                nc.sync.dma_start(out=w1_sb[:, kb * H:(kb + 1) * H],
                                  in_=W1[kb * P:(kb + 1) * P, :])
            w2_sb = constp.tile([H, C], f32, tag="w2")
            nc.sync.dma_start(out=w2_sb[:, :], in_=W2[:, :])
            b1_sb = constp.tile([P, H], f32, tag="b1")
            nc.sync.dma_start(out=b1_sb[:, :], in_=b1b[:, :])
            b2_sb = constp.tile([P, C], f32, tag="b2")
            nc.sync.dma_start(out=b2_sb[:, :], in_=b2b[:, :])
            ident = constp.tile([P, P], f32, tag="ident")
            make_identity(nc, ident[:])

            h1_sb = persist.tile([P, W * H], f32, tag="h1")
            agg_sb = persist.tile([P, W * H], f32, tag="agg")
            o_sb = persist.tile([P, W * C], f32, tag="o")
            e_sb = persist.tile([P, W * C], f32, tag="e")
            red_sb = persist.tile([P, 2 * W], f32, tag="red")

            # ---- Phase 1: h1 = x @ W1, per 128-node window ----
            for wc in range(0, W, XTW):
                nw = min(XTW, W - wc)
                xt_sb = xtp.tile([P, KB, XTW * P], f32, tag="xt")
                for kb in range(KB):
                    nc.sync.dma_start(
                        out=xt_sb[:, kb, : nw * P],
                        in_=xT[kb * P:(kb + 1) * P, wc * P:(wc + nw) * P],
                    )
                for w in range(wc, wc + nw):
                    ph = ps_h.tile([P, H], f32, tag="ph")
                    for kb in range(KB):
                        nc.tensor.matmul(
                            out=ph[:, :],
                            lhsT=xt_sb[:, kb, (w - wc) * P:(w - wc + 1) * P],
                            rhs=w1_sb[:, kb * H:(kb + 1) * H],
                            start=(kb == 0),
                            stop=(kb == KB - 1),
                        )
                    nc.scalar.copy(out=h1_sb[:, w * H:(w + 1) * H], in_=ph[:, :])

            # write shard (row = p*W + w, contiguous per partition) and AllGather
            nc.sync.dma_start(
                out=h1_shard[:, :].rearrange("(p w) h -> p (w h)", p=P),
                in_=h1_sb[:, :],
            )
            if lvl < 1:
                return
            nc.gpsimd.collective_compute(
                "AllGather",
                mybir.AluOpType.bypass,
                replica_groups=[list(range(NCORES))],
                ins=[h1_shard[:, :]],
                outs=[h1_full[:, :]],
            )

            # ---- Phases 2/4: packed edge gather + one-hot weight + reduce ----
            def edge_layer(table, dst_sb, li, only_gather=False):
                table4 = table[:, :].rearrange("(r k) h -> r (k h)", k=4)
                for ci, (w0, w1, off0, S_c) in enumerate(chunks):
                    idx_t = idxmp.tile([P, CS_ALLOC * 8], mybir.dt.int16, tag="idxc")
                    nc.sync.dma_start(out=idx_t[:, : S_c * 8],
                                      in_=idx[:, off0 * 8:(off0 + S_c) * 8])
                    w4_t = w4mp.tile([P, CS_ALLOC * 4], f32, tag="w4c")
                    nc.sync.dma_start(out=w4_t[:, : S_c * 4],
                                      in_=w4[:, off0 * 4:(off0 + S_c) * 4])
                    ga = gathp.tile([P, CS_ALLOC * 64], f32, tag="ga")
                    ca = S_c // 2
                    for hi, (c0, c1) in enumerate(((0, ca), (ca, S_c))):
                        nc.gpsimd.dma_gather(
                            ga[:, c0 * 64: c1 * 64].rearrange(
                                "p (m e) -> p m e", e=64),
                            table4,
                            idx_t[:, c0 * 8: c1 * 8],
                            (c1 - c0) * P, (c1 - c0) * P, 64,
                            queue_num=(2 * (li * len(chunks) + ci) + hi) % NQ,
                            single_packet=False,
                        )
                    if only_gather:
                        continue
                    # msg *= w4 (pack-position one-hot weight, broadcast over H)
                    ga3 = ga[:, : S_c * 64].rearrange("p (m h) -> p m h", h=H)
                    nc.vector.tensor_tensor(
                        out=ga3,
                        in0=ga3,
                        in1=w4_t[:, : S_c * 4].to_broadcast([P, S_c * 4, H]),
                        op=mybir.AluOpType.mult,
                    )
                    for w in range(w0, w1):
                        o = (OFF[w] - off0) * 64
                        nc.vector.tensor_reduce(
                            out=dst_sb[:, w * H:(w + 1) * H],
                            in_=ga[:, o: o + K[w] * 64].rearrange(
                                "p (s h) -> p h s", h=H),
                            axis=mybir.AxisListType.X,
                            op=mybir.AluOpType.add,
                        )

            if lvl < 2:
                return
            if lvl == 2:
                edge_layer(h1_full, agg_sb, 0, only_gather=True)
                return
            edge_layer(h1_full, agg_sb, 0)

            # ---- Phase 3: a1 = relu(agg1 + b1); share and AllGather ----
            agg3 = agg_sb[:, :].rearrange("p (w h) -> p w h", h=H)
            nc.vector.tensor_tensor(
                out=agg3, in0=agg3, in1=bcast_mid(b1_sb[:, :], W),
                op=mybir.AluOpType.add,
            )
            nc.vector.tensor_scalar_max(out=agg_sb[:, :], in0=agg_sb[:, :], scalar1=0.0)
            nc.sync.dma_start(
                out=a1_shard[:, :].rearrange("(p w) h -> p (w h)", p=P),
                in_=agg_sb[:, :],
            )
            if lvl < 4:
                return
            nc.gpsimd.collective_compute(
                "AllGather",
                mybir.AluOpType.bypass,
                replica_groups=[list(range(NCORES))],
                ins=[a1_shard[:, :]],
                outs=[a1_full[:, :]],
            )

            if lvl < 5:
                return
            edge_layer(a1_full, h1_sb, 1)  # reuse h1_sb as G (L2 aggregate)
            if lvl < 6:
                return

            # ---- Phase 5: out = log_softmax(G @ W2 + b2) ----
            for w in range(W):
                pt = ps_t.tile([H, P], f32, tag="pt")
                nc.tensor.transpose(
                    out=pt[:, :], in_=h1_sb[:, w * H:(w + 1) * H], identity=ident[:]
                )
                gt_sb = gtp.tile([H, P], f32, tag="gt")
                nc.scalar.copy(out=gt_sb[:, :], in_=pt[:, :])
                po = ps_o.tile([P, C], f32, tag="po")
                nc.tensor.matmul(
                    out=po[:, :], lhsT=gt_sb[:, :], rhs=w2_sb[:, :],
                    start=True, stop=True,
                )
                nc.scalar.copy(out=o_sb[:, w * C:(w + 1) * C], in_=po[:, :])

            o3 = o_sb[:, :].rearrange("p (w c) -> p w c", c=C)
            nc.vector.tensor_tensor(
                out=o3, in0=o3,
                in1=bcast_mid(b2_sb[:, :], W),
                op=mybir.AluOpType.add,
            )
            rmax = red_sb[:, 0:W]
            rsum = red_sb[:, W:2 * W]
            nc.vector.tensor_reduce(out=rmax, in_=o3, axis=mybir.AxisListType.X,
                                    op=mybir.AluOpType.max)
            nc.vector.tensor_tensor(out=o3, in0=o3,
                                    in1=rmax.to_broadcast([P, W, C]),
                                    op=mybir.AluOpType.subtract)
            nc.scalar.activation(out=e_sb[:, :], in_=o_sb[:, :],
                                 func=mybir.ActivationFunctionType.Exp)
            nc.vector.tensor_reduce(out=rsum,
                                    in_=e_sb[:, :].rearrange("p (w c) -> p w c", c=C),
                                    axis=mybir.AxisListType.X,
                                    op=mybir.AluOpType.add)
            nc.scalar.activation(out=rsum, in_=rsum,
                                 func=mybir.ActivationFunctionType.Ln)
            nc.vector.tensor_tensor(out=o3, in0=o3,
                                    in1=rsum.to_broadcast([P, W, C]),
                                    op=mybir.AluOpType.subtract)
            nc.sync.dma_start(
                out=out[:, :].rearrange("(p w) c -> p (w c)", p=P),
                in_=o_sb[:, :],
            )
    return None


def kernel(x, edge_index, edge_weight, W1, b1, W2, b2):
    import concourse.bacc as bacc
    from concourse.bass_utils import run_bass_kernel_spmd

    x = np.asarray(x, dtype=np.float32)
    W1 = np.asarray(W1, dtype=np.float32)
    b1 = np.asarray(b1, dtype=np.float32)
    W2 = np.asarray(W2, dtype=np.float32)
    b2 = np.asarray(b2, dtype=np.float32)
    edge_weight = np.asarray(edge_weight, dtype=np.float32)
    edge_index = np.asarray(edge_index)

    N = x.shape[0]
    H = W1.shape[1]
    C = W2.shape[1]

    t0 = time.time()
    cfg, idx16, w4d, xts, node_of, out_rows = _host_prep(x, edge_index, edge_weight)
    LAST["prep_s"] = time.time() - t0

    t0 = time.time()
    nc = bacc.Bacc("TRN2", target_bir_lowering=False, debug=False,
                   num_devices=NCORES, num_swdge_queues=NQ)
    _build(nc, cfg, H, C)
    nc.compile()
    LAST["build_s"] = time.time() - t0

    b1b = np.broadcast_to(b1, (P, H)).copy()
    b2b = np.broadcast_to(b2, (P, C)).copy()
    in_maps = [
        {
            "xT": xts[k],
            "W1": W1, "W2": W2, "b1b": b1b, "b2b": b2b,
            "idx": idx16[k], "w4": w4d[k],
        }
        for k in range(NCORES)
    ]

    t0 = time.time()
    res = run_bass_kernel_spmd(
        nc, in_maps, core_ids=list(range(NCORES)),
        trace=bool(int(os.environ.get("GCN_TRACE", "0"))),
    )
    LAST["run_s"] = time.time() - t0
    LAST["results"] = res
    LAST["cfg"] = cfg

    outf = np.empty((N, C), np.float32)
    for k in range(NCORES):
        outf[node_of[:, k]] = res.results[k]["out"][out_rows]
    return outf


# revision 18
# speedup vs baseline: 1.1089x; 1.1089x over previous
"""2-layer GCN (GCNConv without normalization) as a Bass/Trainium2 SPMD kernel on 8 NeuronCores.

Strategy (graph/data parallel, node sharding):
  - Nodes are sorted by in-degree and dealt round-robin to the 8 cores, so
    every core owns ~E/8 edges and windows of 128 nodes have near-uniform
    max in-degree (minimal gather padding).
  - Layer algebra: segment_sum commutes with the linear maps, so both layers
    aggregate 16-wide tables:
        L1: h1 = x @ W1 (16 wide); agg1 = scatter_add(w * h1[src]);
            a1 = relu(agg1 + b1)
        L2: agg2 = scatter_add(w * a1[src]); out = log_softmax(agg2 @ W2 + b2)
  - Edge gather: the 16-wide tables are viewed as 4-node-packed 256B rows
    ([R/4, 64] f32).  Each destination slot fetches its source's pack with a
    single dma_gather descriptor (int16 pack ids < 32768).  Each <=63-column
    chunk is issued as two half-gathers rotated over the 4 SWDGE queues with
    single_packet=False (~500 descriptors/ring keeps 2+ instructions in
    flight per ring; measured ~10x cheaper per edge than the per-column
    indirect_dma_start baseline, which pays ~1.3 us per 128 descriptors).
  - The per-edge weight is stored as a 4-wide one-hot (w at the source's
    position in its pack): one in-place DVE multiply + one strided
    tensor_reduce over 4*K[w] elements per 128-node window performs both the
    pack selection and the segment sum.
  - Gather padding: each destination window of 128 nodes is padded to the
    window's max degree; pad slots use pack 0 with all-zero w4.

The harness calls kernel(**inputs) with full inputs; sharding happens here.
"""

import os
import sys
import time

import numpy as np

sys.path.insert(0, "/opt/trn_rl_repo")

NCORES = 8
P = 128
NQ = 4           # SWDGE queues to rotate dma_gather over
CS_MAX = 63      # slot columns per chunk: 63*128 idx -> 505 descs/ring, 2 fit per 1024-ring

# stash for test harness introspection (exec time, etc.)
LAST = {}


def _host_prep(x, edge_index, edge_weight):
    """Build the sharded/padded data layout. Returns (cfg, per_core_arrays)."""
    N, Fin = x.shape
    E = edge_index.shape[1]
    src = np.ascontiguousarray(edge_index[0]).astype(np.int64)
    dst = np.ascontiguousarray(edge_index[1]).astype(np.int64)

    deg = np.bincount(dst, minlength=N)
    order = np.argsort(-deg, kind="stable")  # node ids, highest in-degree first
    rank_of = np.empty(N, np.int64)
    rank_of[order] = np.arange(N)
    core_of = rank_of % NCORES
    lrank = rank_of // NCORES

    NLOC = -(-N // NCORES)          # nodes per core (assumes N % NCORES == 0 for exactness)
    W = -(-NLOC // P)               # windows per core
    NPAD = W * P
    w_of = lrank // P
    p_of = lrank % P
    # gather-table row of each node (same layout for the h1 and a1 tables)
    tbl = (core_of * NPAD + p_of * W + w_of).astype(np.int64)

    # per-window max degree K[w]: window w covers global degree-ranks [w*1024, ...)
    K = np.zeros(W, np.int64)
    for w in range(W):
        lo = w * P * NCORES
        hi = min((w + 1) * P * NCORES, N)
        K[w] = deg[order[lo:hi]].max() if hi > lo else 0
    K = np.maximum(K, 1)  # keep windows non-empty so every reduce is well-formed
    OFF = np.concatenate([[0], np.cumsum(K)]).astype(np.int64)
    S_total = int(OFF[-1])

    # chunk windows so each chunk's gather fits one dma_gather instruction
    chunks = []
    w0, s0 = 0, 0
    for w in range(W):
        if s0 + K[w] > CS_MAX and w > w0:
            chunks.append((w0, w, int(OFF[w0]), int(s0)))
            w0, s0 = w, 0
        s0 += K[w]
    chunks.append((w0, W, int(OFF[w0]), int(s0)))

    # edge slots: sort edges by destination node id; j-th edge of node d goes
    # to (core_of[d], p_of[d], OFF[w_of[d]] + j)
    e_sort = np.argsort(dst, kind="stable")
    ds = dst[e_sort]
    ss = src[e_sort]
    ws = np.ascontiguousarray(edge_weight)[e_sort].astype(np.float32)
    starts = np.concatenate([[0], np.cumsum(deg)])
    j = np.arange(E) - starts[ds]
    col = OFF[w_of[ds]] + j

    # 4-node pack id + position-in-pack one-hot weights
    pack_arr = np.zeros((NCORES, P, S_total), np.int16)
    pack_arr[core_of[ds], p_of[ds], col] = (tbl[ss] // 4).astype(np.int16)
    w4_arr = np.zeros((NCORES, P, S_total, 4), np.float32)
    w4_arr[core_of[ds], p_of[ds], col, tbl[ss] % 4] = ws

    # idx16: gather position g = col*128 + p reads int16 idx at
    # [band*16 + g%16, g//16]; replicate into all 8 bands (HW reads a
    # queue-dependent band; CoreSim reads band 0).
    NCOLS = S_total * 8
    idx16 = np.empty((NCORES, P, NCOLS), np.int16)
    for k in range(NCORES):
        g_order = pack_arr[k].T.reshape(-1)            # g = col*128 + p
        band = g_order.reshape(NCOLS, 16).T            # [16, NCOLS]
        idx16[k] = np.tile(band, (8, 1))
    w4d = np.ascontiguousarray(w4_arr.reshape(NCORES, P, S_total * 4))

    # x^T shards in lrank order: core k, column l -> node with lrank l
    # (the h1_shard write AP maps window/partition back to table row p*W+w)
    node_of = order[: NLOC * NCORES].reshape(NLOC, NCORES)
    xts = np.zeros((NCORES, Fin, NPAD), np.float32)
    for k in range(NCORES):
        xts[k, :, :NLOC] = x[node_of[:, k]].T

    # output reshuffle: result row p*W+w of core k -> node node_of[w*128+p, k]
    l = np.arange(NLOC)
    out_rows = (l % P) * W + l // P  # row in kernel output for local slot l

    cfg = dict(
        N=N, Fin=Fin, E=E, NLOC=NLOC, W=W, NPAD=NPAD,
        K=K.tolist(), OFF=OFF.tolist(), S_total=S_total, chunks=chunks,
    )
    return cfg, idx16, w4d, xts, node_of, out_rows


def _build(nc, cfg, H, C):
    """Emit the Bass/Tile program (identical on all cores).

    GCN_STAGE env limits how much of the pipeline is emitted (debug bisect):
      h1 < ag1 < g1 < l1 < ag2 < l2 < full (default)
    """
    import concourse.mybir as mybir
    import concourse.tile as tile
    from concourse import bass
    from concourse.masks import make_identity

    Fin, W, NPAD, S_total = cfg["Fin"], cfg["W"], cfg["NPAD"], cfg["S_total"]
    K, OFF, chunks = cfg["K"], cfg["OFF"], cfg["chunks"]
    KB = Fin // P  # K-blocks for the x @ W1 matmul
    f32 = mybir.dt.float32

    xT = nc.dram_tensor("xT", [Fin, NPAD], f32, kind="ExternalInput")
    W1 = nc.dram_tensor("W1", [Fin, H], f32, kind="ExternalInput")
    W2 = nc.dram_tensor("W2", [H, C], f32, kind="ExternalInput")
    b1b = nc.dram_tensor("b1b", [P, H], f32, kind="ExternalInput")
    b2b = nc.dram_tensor("b2b", [P, C], f32, kind="ExternalInput")
    idx = nc.dram_tensor("idx", [P, S_total * 8], mybir.dt.int16, kind="ExternalInput")
    w4 = nc.dram_tensor("w4", [P, S_total * 4], f32, kind="ExternalInput")
    out = nc.dram_tensor("out", [NPAD, C], f32, kind="ExternalOutput")

    h1_shard = nc.dram_tensor("h1_shard", [NPAD, H], f32)
    h1_full = nc.dram_tensor("h1_full", [NCORES * NPAD, H], f32, addr_space="Shared")
    a1_shard = nc.dram_tensor("a1_shard", [NPAD, H], f32)
    a1_full = nc.dram_tensor("a1_full", [NCORES * NPAD, H], f32, addr_space="Shared")

    XTW = 4  # windows per xT load chunk
    STAGE = os.environ.get("GCN_STAGE", "full")
    ORDER = ["h1", "ag1", "g1", "l1", "ag2", "l2", "full"]
    lvl = ORDER.index(STAGE)

    CS_ALLOC = max(c[3] for c in chunks)  # largest chunk (a lone window may exceed CS_MAX)

    def bcast_mid(ap, n):
        """[P, F] -> [P, n, F] with a step-0 middle dim."""
        return bass.AP(ap.tensor, ap.offset, [list(ap.ap[0]), [0, n], list(ap.ap[1])])

    GBUFS = 9 if CS_ALLOC <= 66 else 5  # keep the gather pool inside SBUF
    with tile.TileContext(nc) as tc:
        with (
            tc.tile_pool(name="const", bufs=1) as constp,
            tc.tile_pool(name="persist", bufs=1) as persist,
            tc.tile_pool(name="xt", bufs=2) as xtp,
            tc.tile_pool(name="gath", bufs=GBUFS) as gathp,
            tc.tile_pool(name="idxm", bufs=8) as idxmp,
            tc.tile_pool(name="w4m", bufs=8) as w4mp,
            tc.tile_pool(name="gt", bufs=2) as gtp,
            tc.tile_pool(name="ps_h", bufs=2, space="PSUM") as ps_h,
            tc.tile_pool(name="ps_t", bufs=2, space="PSUM") as ps_t,
            tc.tile_pool(name="ps_o", bufs=2, space="PSUM") as ps_o,
        ):
            # constants
            w1_sb = constp.tile([P, KB * H], f32, tag="w1")
            for kb in range(KB):
                nc.sync.dma_start(out=w1_sb[:, kb * H:(kb + 1) * H],
                                  in_=W1[kb * P:(kb + 1) * P, :])
            w2_sb = constp.tile([H, C], f32, tag="w2")
            nc.sync.dma_start(out=w2_sb[:, :], in_=W2[:, :])
            b1_sb = constp.tile([P, H], f32, tag="b1")
            nc.sync.dma_start(out=b1_sb[:, :], in_=b1b[:, :])
            b2_sb = constp.tile([P, C], f32, tag="b2")
            nc.sync.dma_start(out=b2_sb[:, :], in_=b2b[:, :])
            ident = constp.tile([P, P], f32, tag="ident")
            make_identity(nc, ident[:])

            h1_sb = persist.tile([P, W * H], f32, tag="h1")
            agg_sb = persist.tile([P, W * H], f32, tag="agg")
            o_sb = persist.tile([P, W * C], f32, tag="o")
            e_sb = persist.tile([P, W * C], mybir.dt.bfloat16, tag="e")
            red_sb = persist.tile([P, 2 * W], f32, tag="red")

            # ---- Phase 1: h1 = x @ W1, per 128-node window ----
            for wc in range(0, W, XTW):
                nw = min(XTW, W - wc)
                xt_sb = xtp.tile([P, KB, XTW * P], f32, tag="xt")
                for kb in range(KB):
                    nc.sync.dma_start(
                        out=xt_sb[:, kb, : nw * P],
                        in_=xT[kb * P:(kb + 1) * P, wc * P:(wc + nw) * P],
                    )
                for w in range(wc, wc + nw):
                    ph = ps_h.tile([P, H], f32, tag="ph")
                    for kb in range(KB):
                        nc.tensor.matmul(
                            out=ph[:, :],
                            lhsT=xt_sb[:, kb, (w - wc) * P:(w - wc + 1) * P],
                            rhs=w1_sb[:, kb * H:(kb + 1) * H],
                            start=(kb == 0),
                            stop=(kb == KB - 1),
                        )
                    nc.scalar.copy(out=h1_sb[:, w * H:(w + 1) * H], in_=ph[:, :])

            # write shard (row = p*W + w, contiguous per partition) and AllGather
            nc.sync.dma_start(
                out=h1_shard[:, :].rearrange("(p w) h -> p (w h)", p=P),
                in_=h1_sb[:, :],
            )
            if lvl < 1:
                return
            nc.gpsimd.collective_compute(
                "AllGather",
                mybir.AluOpType.bypass,
                replica_groups=[list(range(NCORES))],
                ins=[h1_shard[:, :]],
                outs=[h1_full[:, :]],
            )

            # ---- Phases 2/4: packed edge gather + one-hot weight + reduce ----
            def edge_layer(table, dst_sb, li, only_gather=False):
                table4 = table[:, :].rearrange("(r k) h -> r (k h)", k=4)
                for ci, (w0, w1, off0, S_c) in enumerate(chunks):
                    idx_t = idxmp.tile([P, CS_ALLOC * 8], mybir.dt.int16, tag="idxc")
                    nc.sync.dma_start(out=idx_t[:, : S_c * 8],
                                      in_=idx[:, off0 * 8:(off0 + S_c) * 8])
                    w4_t = w4mp.tile([P, CS_ALLOC * 4], f32, tag="w4c")
                    nc.sync.dma_start(out=w4_t[:, : S_c * 4],
                                      in_=w4[:, off0 * 4:(off0 + S_c) * 4])
                    ga = gathp.tile([P, CS_ALLOC * 64], f32, tag="ga")
                    ca = S_c // 2
                    for hi, (c0, c1) in enumerate(((0, ca), (ca, S_c))):
                        nc.gpsimd.dma_gather(
                            ga[:, c0 * 64: c1 * 64].rearrange(
                                "p (m e) -> p m e", e=64),
                            table4,
                            idx_t[:, c0 * 8: c1 * 8],
                            (c1 - c0) * P, (c1 - c0) * P, 64,
                            queue_num=(2 * (li * len(chunks) + ci) + hi) % NQ,
                            single_packet=False,
                        )
                    if only_gather:
                        continue
                    # msg *= w4 (pack-position one-hot weight, broadcast over H)
                    ga3 = ga[:, : S_c * 64].rearrange("p (m h) -> p m h", h=H)
                    nc.vector.tensor_tensor(
                        out=ga3,
                        in0=ga3,
                        in1=w4_t[:, : S_c * 4].to_broadcast([P, S_c * 4, H]),
                        op=mybir.AluOpType.mult,
                    )
                    for w in range(w0, w1):
                        o = (OFF[w] - off0) * 64
                        nc.vector.tensor_reduce(
                            out=dst_sb[:, w * H:(w + 1) * H],
                            in_=ga[:, o: o + K[w] * 64].rearrange(
                                "p (s h) -> p h s", h=H),
                            axis=mybir.AxisListType.X,
                            op=mybir.AluOpType.add,
                        )

            if lvl < 2:
                return
            if lvl == 2:
                edge_layer(h1_full, agg_sb, 0, only_gather=True)
                return
            edge_layer(h1_full, agg_sb, 0)

            # ---- Phase 3: a1 = relu(agg1 + b1); share and AllGather ----
            agg3 = agg_sb[:, :].rearrange("p (w h) -> p w h", h=H)
            nc.vector.tensor_tensor(
                out=agg3, in0=agg3, in1=bcast_mid(b1_sb[:, :], W),
                op=mybir.AluOpType.add,
            )
            nc.vector.tensor_scalar_max(out=agg_sb[:, :], in0=agg_sb[:, :], scalar1=0.0)
            nc.sync.dma_start(
                out=a1_shard[:, :].rearrange("(p w) h -> p (w h)", p=P),
                in_=agg_sb[:, :],
            )
            if lvl < 4:
                return
            nc.gpsimd.collective_compute(
                "AllGather",
                mybir.AluOpType.bypass,
                replica_groups=[list(range(NCORES))],
                ins=[a1_shard[:, :]],
                outs=[a1_full[:, :]],
            )

            if lvl < 5:
                return
            edge_layer(a1_full, h1_sb, 1)  # reuse h1_sb as G (L2 aggregate)
            if lvl < 6:
                return

            # ---- Phase 5: out = log_softmax(G @ W2 + b2) ----
            for w in range(W):
                pt = ps_t.tile([H, P], f32, tag="pt")
                nc.tensor.transpose(
                    out=pt[:, :], in_=h1_sb[:, w * H:(w + 1) * H], identity=ident[:]
                )
                gt_sb = gtp.tile([H, P], f32, tag="gt")
                nc.scalar.copy(out=gt_sb[:, :], in_=pt[:, :])
                po = ps_o.tile([P, C], f32, tag="po")
                nc.tensor.matmul(
                    out=po[:, :], lhsT=gt_sb[:, :], rhs=w2_sb[:, :],
                    start=True, stop=True,
                )
                nc.scalar.copy(out=o_sb[:, w * C:(w + 1) * C], in_=po[:, :])

            o3 = o_sb[:, :].rearrange("p (w c) -> p w c", c=C)
            nc.vector.tensor_tensor(
                out=o3, in0=o3,
                in1=bcast_mid(b2_sb[:, :], W),
                op=mybir.AluOpType.add,
            )
            rmax = red_sb[:, 0:W]
            rsum = red_sb[:, W:2 * W]
            nc.vector.tensor_reduce(out=rmax, in_=o3, axis=mybir.AxisListType.X,
                                    op=mybir.AluOpType.max)
            nc.vector.tensor_tensor(out=o3, in0=o3,
                                    in1=rmax.to_broadcast([P, W, C]),
                                    op=mybir.AluOpType.subtract)
            nc.scalar.activation(out=e_sb[:, :], in_=o_sb[:, :],
                                 func=mybir.ActivationFunctionType.Exp)
            nc.vector.tensor_reduce(out=rsum,
                                    in_=e_sb[:, :].rearrange("p (w c) -> p w c", c=C),
                                    axis=mybir.AxisListType.X,
                                    op=mybir.AluOpType.add)
            nc.scalar.activation(out=rsum, in_=rsum,
                                 func=mybir.ActivationFunctionType.Ln)
            nc.vector.tensor_tensor(out=o3, in0=o3,
                                    in1=rsum.to_broadcast([P, W, C]),
                                    op=mybir.AluOpType.subtract)
            nc.sync.dma_start(
                out=out[:, :].rearrange("(p w) c -> p (w c)", p=P),
                in_=o_sb[:, :],
            )
    return None


def kernel(x, edge_index, edge_weight, W1, b1, W2, b2):
    import concourse.bacc as bacc
    from concourse.bass_utils import run_bass_kernel_spmd

    x = np.asarray(x, dtype=np.float32)
    W1 = np.asarray(W1, dtype=np.float32)
    b1 = np.asarray(b1, dtype=np.float32)
    W2 = np.asarray(W2, dtype=np.float32)
    b2 = np.asarray(b2, dtype=np.float32)
    edge_weight = np.asarray(edge_weight, dtype=np.float32)
    edge_index = np.asarray(edge_index)

    N = x.shape[0]
    H = W1.shape[1]
    C = W2.shape[1]

    t0 = time.time()
    cfg, idx16, w4d, xts, node_of, out_rows = _host_prep(x, edge_index, edge_weight)
    LAST["prep_s"] = time.time() - t0

    t0 = time.time()
    nc = bacc.Bacc("TRN2", target_bir_lowering=False, debug=False,
                   num_devices=NCORES, num_swdge_queues=NQ)
    _build(nc, cfg, H, C)
    nc.compile()
    LAST["build_s"] = time.time() - t0

    b1b = np.broadcast_to(b1, (P, H)).copy()
    b2b = np.broadcast_to(b2, (P, C)).copy()
    in_maps = [
        {
            "xT": xts[k],
            "W1": W1, "W2": W2, "b1b": b1b, "b2b": b2b,
            "idx": idx16[k], "w4": w4d[k],
        }
        for k in range(NCORES)
    ]

    t0 = time.time()
    res = run_bass_kernel_spmd(
        nc, in_maps, core_ids=list(range(NCORES)),
        trace=bool(int(os.environ.get("GCN_TRACE", "0"))),
    )
    LAST["run_s"] = time.time() - t0
    LAST["results"] = res
    LAST["cfg"] = cfg

    outf = np.empty((N, C), np.float32)
    for k in range(NCORES):
        outf[node_of[:, k]] = res.results[k]["out"][out_rows]
    return outf
